# Initial kernel scaffold
#
"""DisConv GNN message-passing kernel for 8 Trainium2 NeuronCores.

Problem: Z = l2norm(features @ W_k + b_k); 4 iterations of
  att[k] = softmax_k(mask * (Z_k Z_k^T)); Z = l2norm(Z + att @ Z)
Output: [N, K*D] channel-concat.

Strategy (row sharding, N=2048 over 8 cores, 256 rows each):
- Each core holds the full replicated Z in bf16 in two layouts:
  ZT (channel-major [32c x 2048n] stacks of 4 channels) for score matmuls,
  Znm (n-major [128, 16blk*8k*32c]) for aggregation matmuls.
  The core's own 256-column f32 state never leaves the core.
- Per m-block [128m x 256n]: 8 score matmuls (D=32 contraction), one exp
  per channel-stack (ScalarE), bf16 pairwise-tree denominator, approx
  reciprocal, mask*recip, one broadcast multiply -> att; 8 col-packed
  aggregation matmuls accumulate over the 16 m-blocks in PSUM.
- Softmax restructuring: softmax input masking is k-independent, so
  att = mask * exp(S) / sum_k exp(S) exactly matches the reference.
- l2norm via rnorm = exp(-0.5*ln(s^2+eps)) (Ln+Exp share an ACT table set).
- Between iterations: AllGather of 256KB/rank bf16 (both layouts).
"""

import sys

sys.path.insert(0, "/opt/trn_rl_repo")

import numpy as np
import ml_dtypes

N = 2048
IN_DIM = 128
K = 8
D = 32
ITERS = 4
NCORES = 8
NLOC = N // NCORES  # 256
NBLK = N // 128  # 16
EPS2 = 1e-24

BF = ml_dtypes.bfloat16

_compiled = None


def _build():
    import concourse.bacc as bacc
    import concourse.mybir as mybir
    from concourse import tile

    f32 = mybir.dt.float32
    bf16 = mybir.dt.bfloat16
    AF = mybir.ActivationFunctionType
    ALU = mybir.AluOpType

    nc = bacc.Bacc("TRN2", target_bir_lowering=False, debug=False, num_devices=NCORES)

    # ---- I/O -------------------------------------------------------------
    featT_loc = nc.dram_tensor("featT_loc", [IN_DIM, NLOC], f32, kind="ExternalInput")
    maskT_in = nc.dram_tensor("maskT", [NBLK, 128, NLOC], bf16, kind="ExternalInput")
    wstack_in = nc.dram_tensor("wstack", [IN_DIM, K * D], f32, kind="ExternalInput")
    bstack_in = nc.dram_tensor("bstack", [128, 2], f32, kind="ExternalInput")
    onesblk_in = nc.dram_tensor("onesblk", [128, 128], f32, kind="ExternalInput")
    id128_in = nc.dram_tensor("id128", [128, 128], f32, kind="ExternalInput")
    out_dram = nc.dram_tensor("out", [2, 128, NLOC], f32, kind="ExternalOutput")

    rg = [list(range(NCORES))]

    with tile.TileContext(nc) as tc:
        with (
            tc.tile_pool(name="const", bufs=1) as constp,
            tc.tile_pool(name="state", bufs=2) as statep,
            tc.tile_pool(name="work", bufs=2) as workp,
            tc.tile_pool(name="psum", bufs=3, space="PSUM") as psp,
            tc.tile_pool(name="psagg", bufs=2, space="PSUM") as psaggp,
            tc.tile_pool(name="dram", bufs=2, space="DRAM") as dramp,
        ):
            # ---- persistent SBUF tensors --------------------------------
            featT = constp.tile([IN_DIM, NLOC], f32)
            nc.sync.dma_start(featT[:], featT_loc[:])
            wstack = constp.tile([IN_DIM, K * D], f32)
            nc.sync.dma_start(wstack[:], wstack_in[:])
            bstack = constp.tile([128, 2], f32)
            nc.sync.dma_start(bstack[:], bstack_in[:])
            onesblk = constp.tile([128, 128], f32)
            nc.sync.dma_start(onesblk[:], onesblk_in[:])
            id128 = constp.tile([128, 128], f32)
            nc.sync.dma_start(id128[:], id128_in[:])
            maskT = constp.tile([128, NBLK * NLOC], bf16)
            for q in range(4):
                nc.sync.dma_start(
                    maskT[:, q * 4 * NLOC : (q + 1) * 4 * NLOC].rearrange(
                        "p (b n) -> p b n", b=4
                    ),
                    maskT_in[4 * q : 4 * q + 4].rearrange("b p n -> p b n"),
                )

            # replicated Z (bf16, rebuilt each round via AllGather)
            ZTs = [constp.tile([128, N], bf16, name=f"ZT{s}") for s in range(2)]
            Znm = constp.tile([128, NBLK * K * D], bf16)

            def normalize_and_distribute(zsum, rnd, last):
                """zsum: 2 stacks [128(4ch x 32c), NLOC] f32 (SBUF or PSUM src
                handled by caller adds). Produces:
                - new local f32 state (returned)
                - bf16 local tiles in both layouts, AllGather, reload replicas
                or, if last, writes the output DRAM tensor."""
                zloc = []
                for s in range(2):
                    sq = workp.tile([128, NLOC], f32, name=f"sq{rnd}{s}", tag="sq")
                    nc.vector.tensor_tensor(sq[:], zsum[s][:], zsum[s][:], ALU.mult)
                    n2 = psp.tile([128, NLOC], f32, name=f"n2{rnd}{s}", tag="sps")
                    nc.tensor.matmul(n2[:], onesblk[:], sq[:], start=True, stop=True)
                    lg = workp.tile([128, NLOC], f32, name=f"lg{rnd}{s}", tag="lg")
                    nc.scalar.activation(lg[:], n2[:], AF.Ln, bias=EPS2)
                    rn = workp.tile([128, NLOC], f32, name=f"rn{rnd}{s}", tag="rn")
                    nc.scalar.activation(rn[:], lg[:], AF.Exp, scale=-0.5)
                    zn = statep.tile([128, NLOC], f32, name=f"zloc{rnd}{s}", tag=f"zloc{s}")
                    nc.vector.tensor_tensor(zn[:], zsum[s][:], rn[:], ALU.mult)
                    zloc.append(zn)

                # transpose local columns to n-major: 2 chunks of 128 rows
                pst = []
                for c in range(2):
                    pt = psp.tile([128, 256], f32, name=f"pt{rnd}{c}", tag="sps")
                    for s in range(2):
                        for i in range(4):
                            nc.tensor.transpose(
                                pt[:, s * 128 + i * 32 : s * 128 + (i + 1) * 32],
                                zloc[s][32 * i : 32 * (i + 1), c * 128 : (c + 1) * 128],
                                id128[32 * i : 32 * (i + 1), 32 * i : 32 * (i + 1)],
                                tile_position=(32 * i, 0),
                            )
                    pst.append(pt)

                if last:
                    for c in range(2):
                        ot = workp.tile([128, 256], f32, name=f"ot{c}", tag="ot")
                        nc.scalar.copy(ot[:], pst[c][:])
                        nc.sync.dma_start(out_dram[c], ot[:])
                    return zloc

                # bf16 casts
                ztl = []
                for s in range(2):
                    t = statep.tile([128, NLOC], bf16, name=f"ztl{rnd}{s}", tag=f"ztl{s}")
                    nc.vector.tensor_copy(t[:], zloc[s][:])
                    ztl.append(t)
                znml = []
                for c in range(2):
                    t = workp.tile([128, 256], bf16, name=f"znml{rnd}{c}", tag=f"znml{c}")
                    nc.vector.tensor_copy(t[:], pst[c][:])
                    znml.append(t)

                # AllGather both layouts (4 x [128, 256] bf16 = 256KB/rank)
                agin = dramp.tile([4, 128, 256], bf16, name=f"agin{rnd}", tag="agin")
                agout = dramp.tile(
                    [NCORES, 4, 128, 256], bf16,
                    name=f"agout{rnd}", tag="agout", addr_space="Shared",
                )
                for s in range(2):
                    nc.sync.dma_start(agin[s], ztl[s][:])
                for c in range(2):
                    nc.sync.dma_start(agin[2 + c], znml[c][:])
                nc.gpsimd.collective_compute(
                    "AllGather",
                    mybir.AluOpType.bypass,
                    replica_groups=rg,
                    ins=[agin[:].opt()],
                    outs=[agout[:].opt()],
                )
                # reload replicas
                for s in range(2):
                    for r in range(NCORES):
                        nc.sync.dma_start(
                            ZTs[s][:, r * NLOC : (r + 1) * NLOC], agout[r, s]
                        )
                for r in range(NCORES):
                    for c in range(2):
                        blk = r * 2 + c
                        nc.sync.dma_start(
                            Znm[:, blk * 256 : (blk + 1) * 256], agout[r, 2 + c]
                        )
                return zloc, ztl

            # ---- init: Z0 = l2norm(features @ W + b) for local columns ---
            zsum0 = []
            for s in range(2):
                ip = psp.tile([128, NLOC], f32, name=f"initp{s}", tag="sps")
                for i in range(4):
                    nc.tensor.matmul(
                        ip[32 * i : 32 * (i + 1), :],
                        wstack[:, (4 * s + i) * D : (4 * s + i + 1) * D],
                        featT[:],
                        start=True,
                        stop=True,
                        tile_position=(0, 32 * i),
                    )
                zs = workp.tile([128, NLOC], f32, name=f"zsum0{s}", tag="zsum")
                nc.vector.tensor_scalar(
                    zs[:], ip[:], bstack[:, s : s + 1], None, ALU.add
                )
                zsum0.append(zs)
            zloc, ztl = normalize_and_distribute(zsum0, 0, last=False)

            # ---- iterations ---------------------------------------------
            for it in range(ITERS):
                aggps = [
                    psaggp.tile([128, NLOC], f32, name=f"agg{it}{s}", tag=f"agg{s}")
                    for s in range(2)
                ]
                for blk in range(NBLK):
                    sps = [
                        psp.tile([128, 4 * NLOC], f32, name=f"sps{it}{blk}{s}", tag="sps")
                        for s in range(2)
                    ]
                    for s in range(2):
                        for i in range(4):
                            nc.tensor.matmul(
                                sps[s][:, i * NLOC : (i + 1) * NLOC],
                                ZTs[s][32 * i : 32 * (i + 1), blk * 128 : (blk + 1) * 128],
                                ztl[s][32 * i : 32 * (i + 1), :],
                                start=True,
                                stop=True,
                                tile_position=(32 * i, 0),
                            )
                    eall = workp.tile([128, K * NLOC], bf16, name=f"eall{it}{blk}", tag="eall")
                    for s in range(2):
                        nc.scalar.activation(
                            eall[:, s * 1024 : (s + 1) * 1024], sps[s][:], AF.Exp
                        )
                    # denominator tree: 3 ops
                    t1 = workp.tile([128, 1024], bf16, name=f"t1_{it}{blk}", tag="t1")
                    ev = eall[:].rearrange("p (a n) -> p a n", a=4)
                    t1v = t1[:].rearrange("p (a n) -> p a n", a=4)
                    nc.vector.tensor_tensor(
                        t1v, ev[:, :, 0:NLOC], ev[:, :, NLOC : 2 * NLOC], ALU.add
                    )
                    t2 = workp.tile([128, 512], bf16, name=f"t2_{it}{blk}", tag="t2")
                    t1w = t1[:].rearrange("p (a n) -> p a n", a=2)
                    t2v = t2[:].rearrange("p (a n) -> p a n", a=2)
                    nc.vector.tensor_tensor(
                        t2v, t1w[:, :, 0:NLOC], t1w[:, :, NLOC : 2 * NLOC], ALU.add
                    )
                    den = workp.tile([128, NLOC], f32, name=f"den{it}{blk}", tag="den")
                    nc.vector.tensor_tensor(
                        den[:], t2[:, 0:NLOC], t2[:, NLOC : 2 * NLOC], ALU.add
                    )
                    rcp = workp.tile([128, NLOC], f32, name=f"rcp{it}{blk}", tag="rcp")
                    nc.vector.reciprocal_approx_fast(rcp[:], den[:])
                    rmask = workp.tile([128, NLOC], bf16, name=f"rm{it}{blk}", tag="rm")
                    nc.vector.tensor_tensor(
                        rmask[:], rcp[:], maskT[:, blk * NLOC : (blk + 1) * NLOC], ALU.mult
                    )
                    att = workp.tile([128, K * NLOC], bf16, name=f"att{it}{blk}", tag="att")
                    nc.vector.tensor_tensor(
                        att[:].rearrange("p (a n) -> p a n", a=K),
                        eall[:].rearrange("p (a n) -> p a n", a=K),
                        rmask[:, None, :].to_broadcast((128, K, NLOC)),
                        ALU.mult,
                    )
                    for s in range(2):
                        for i in range(4):
                            k = 4 * s + i
                            nc.tensor.matmul(
                                aggps[s][32 * i : 32 * (i + 1), :],
                                Znm[:, (blk * K + k) * D : (blk * K + k + 1) * D],
                                att[:, k * NLOC : (k + 1) * NLOC],
                                start=(blk == 0),
                                stop=(blk == NBLK - 1),
                                tile_position=(0, 32 * i),
                            )
                # residual + renorm + redistribute
                zsum = []
                for s in range(2):
                    zs = workp.tile([128, NLOC], f32, name=f"zsum{it}{s}", tag="zsum")
                    nc.vector.tensor_tensor(zs[:], zloc[s][:], aggps[s][:], ALU.add)
                    zsum.append(zs)
                if it == ITERS - 1:
                    normalize_and_distribute(zsum, it + 1, last=True)
                else:
                    zloc, ztl = normalize_and_distribute(zsum, it + 1, last=False)

    nc.compile()
    return nc


def _prep_inputs(adj, features, W, b):
    adj = np.asarray(adj)
    features = np.asarray(features, np.float32)
    W = np.asarray(W, np.float32)
    b = np.asarray(b, np.float32)

    wstack = np.ascontiguousarray(W.transpose(1, 0, 2).reshape(IN_DIM, K * D))
    bstack = np.zeros((128, 2), np.float32)
    for s in range(2):
        for i in range(4):
            bstack[32 * i : 32 * (i + 1), s] = b[4 * s + i]
    onesblk = np.zeros((128, 128), np.float32)
    for j in range(4):
        onesblk[32 * j : 32 * (j + 1), 32 * j : 32 * (j + 1)] = 1.0
    id128 = np.eye(128, dtype=np.float32)

    in_maps = []
    for c in range(NCORES):
        rows = slice(c * NLOC, (c + 1) * NLOC)
        featT_loc = np.ascontiguousarray(features[rows].T)
        maskT = (adj[rows].T > 0).astype(np.float32).astype(BF)
        maskT = np.ascontiguousarray(maskT.reshape(NBLK, 128, NLOC))
        in_maps.append(
            {
                "featT_loc": featT_loc,
                "maskT": maskT,
                "wstack": wstack,
                "bstack": bstack,
                "onesblk": onesblk,
                "id128": id128,
            }
        )
    return in_maps


def run(adj, features, W, b, trace=False, **trace_kwargs):
    global _compiled
    if _compiled is None:
        _compiled = _build()
    from concourse import bass_utils

    in_maps = _prep_inputs(adj, features, W, b)
    res = bass_utils.run_bass_kernel_spmd(
        _compiled, in_maps, core_ids=list(range(NCORES)), trace=trace, **trace_kwargs
    )
    outs = [res.results[c]["out"].reshape(NLOC, NLOC) for c in range(NCORES)]
    full = np.concatenate(outs, axis=0)
    return full, res


def kernel(adj, features, W, b):
    full, _ = run(adj, features, W, b, trace=False)
    return full


# revision 15
# speedup vs baseline: 1.0948x; 1.0948x over previous
"""DisConv GNN message-passing kernel for 8 Trainium2 NeuronCores.

Problem: Z = l2norm(features @ W_k + b_k); 4 iterations of
  att[k] = softmax_k(mask * (Z_k Z_k^T)); Z = l2norm(Z + att @ Z)
Output: [N, K*D] channel-concat.

Strategy (row sharding, N=2048 over 8 cores, 256 rows each):
- Each core holds the full replicated Z in bf16 in two layouts:
  ZT (channel-major [32c x 2048n] stacks of 4 channels) for score matmuls,
  Znm (n-major [128, 16blk*8k*32c]) for aggregation matmuls.
  The core's own 256-column f32 state never leaves the core.
- Per m-block [128m x 256n]: 8 score matmuls (D=32 contraction), one exp
  per channel-stack (ScalarE), bf16 pairwise-tree denominator, approx
  reciprocal, mask*recip, one broadcast multiply -> att; 8 col-packed
  aggregation matmuls accumulate over the 16 m-blocks in PSUM.
- Softmax restructuring: softmax input masking is k-independent, so
  att = mask * exp(S) / sum_k exp(S) exactly matches the reference.
- l2norm via rnorm = exp(-0.5*ln(s^2+eps)) (Ln+Exp share an ACT table set).
- Between iterations: AllGather of 256KB/rank bf16 (both layouts).
"""

import sys

sys.path.insert(0, "/opt/trn_rl_repo")

import numpy as np
import ml_dtypes

N = 2048
IN_DIM = 128
K = 8
D = 32
ITERS = 4
NCORES = 8
NLOC = N // NCORES  # 256
NBLK = N // 128  # 16
EPS2 = 1e-24

BF = ml_dtypes.bfloat16

_compiled = None


def _build(reps=1):
    import concourse.bacc as bacc
    import concourse.mybir as mybir
    from concourse import tile

    f32 = mybir.dt.float32
    bf16 = mybir.dt.bfloat16
    AF = mybir.ActivationFunctionType
    ALU = mybir.AluOpType

    nc = bacc.Bacc("TRN2", target_bir_lowering=False, debug=False, num_devices=NCORES)

    # ---- I/O -------------------------------------------------------------
    featT_loc = nc.dram_tensor("featT_loc", [IN_DIM, NLOC], f32, kind="ExternalInput")
    maskT_in = nc.dram_tensor("maskT", [NBLK, 128, NLOC], bf16, kind="ExternalInput")
    wstack_in = nc.dram_tensor("wstack", [IN_DIM, K * D], f32, kind="ExternalInput")
    bstack_in = nc.dram_tensor("bstack", [128, 2], f32, kind="ExternalInput")
    onesblk_in = nc.dram_tensor("onesblk", [128, 128], f32, kind="ExternalInput")
    id128_in = nc.dram_tensor("id128", [128, 128], f32, kind="ExternalInput")
    out_dram = nc.dram_tensor("out", [2, 128, NLOC], f32, kind="ExternalOutput")

    rg = [list(range(NCORES))]

    with tile.TileContext(nc) as tc:
        with (
            tc.tile_pool(name="const", bufs=1) as constp,
            tc.tile_pool(name="state", bufs=2) as statep,
            tc.tile_pool(name="work", bufs=2) as workp,
            tc.tile_pool(name="psum", bufs=1, space="PSUM") as psp,
            tc.tile_pool(name="psagg", bufs=1, space="PSUM") as psaggp,
            tc.tile_pool(name="dram", bufs=2, space="DRAM") as dramp,
        ):
            # ---- persistent SBUF tensors --------------------------------
            featT = constp.tile([IN_DIM, NLOC], f32)
            nc.sync.dma_start(featT[:], featT_loc[:])
            wstack = constp.tile([IN_DIM, K * D], f32)
            nc.sync.dma_start(wstack[:], wstack_in[:])
            bstack = constp.tile([128, 2], f32)
            nc.sync.dma_start(bstack[:], bstack_in[:])
            onesblk = constp.tile([128, 128], f32)
            nc.sync.dma_start(onesblk[:], onesblk_in[:])
            id128 = constp.tile([128, 128], f32)
            nc.sync.dma_start(id128[:], id128_in[:])
            epsb = constp.tile([128, 1], f32)
            nc.any.memset(epsb[:], EPS2)
            maskT = constp.tile([128, NBLK * NLOC], bf16)
            for q in range(4):
                nc.sync.dma_start(
                    maskT[:, q * 4 * NLOC : (q + 1) * 4 * NLOC].rearrange(
                        "p (b n) -> p b n", b=4
                    ),
                    maskT_in[4 * q : 4 * q + 4].rearrange("b p n -> p b n"),
                )

            # replicated Z (bf16, rebuilt each round via AllGather)
            ZTs = [constp.tile([128, N], bf16, name=f"ZT{s}") for s in range(2)]
            Znm = constp.tile([128, NBLK * K * D], bf16)

            def normalize_and_distribute(zsum, rnd, last):
                """zsum: 2 stacks [128(4ch x 32c), NLOC] f32 (SBUF or PSUM src
                handled by caller adds). Produces:
                - new local f32 state (returned)
                - bf16 local tiles in both layouts, AllGather, reload replicas
                or, if last, writes the output DRAM tensor."""
                zloc = []
                for s in range(2):
                    sq = workp.tile([128, NLOC], f32, name=f"sq{rnd}{s}", tag="sq")
                    nc.vector.tensor_tensor(sq[:], zsum[s][:], zsum[s][:], ALU.mult)
                    n2 = psp.tile([128, NLOC], f32, name=f"n2{rnd}{s}", tag=f"sps{s}")
                    nc.tensor.matmul(n2[:], onesblk[:], sq[:], start=True, stop=True)
                    lg = workp.tile([128, NLOC], f32, name=f"lg{rnd}{s}", tag="lg")
                    nc.scalar.activation(lg[:], n2[:], AF.Ln, bias=epsb[:])
                    rn = workp.tile([128, NLOC], f32, name=f"rn{rnd}{s}", tag="rn")
                    nc.scalar.activation(rn[:], lg[:], AF.Exp, scale=-0.5)
                    zn = statep.tile([128, NLOC], f32, name=f"zloc{rnd}{s}", tag=f"zloc{s}")
                    nc.vector.tensor_tensor(zn[:], zsum[s][:], rn[:], ALU.mult)
                    zloc.append(zn)

                # transpose local columns to n-major: 2 chunks of 128 rows.
                # Row-group-concurrent PE ops must write different PSUM banks,
                # so each row group i gets its own one-bank tile; stack s picks
                # the column half. Channel 4s+i lands at slot 2i+s ("slot
                # order"), which the E/att/Znm replica layouts share.
                pst = []
                for c in range(2):
                    pt = [
                        psp.tile([128, 64], f32, name=f"pt{rnd}{c}{i}", tag=f"sps{i}")
                        for i in range(4)
                    ]
                    for s in range(2):
                        for i in range(4):
                            nc.tensor.transpose(
                                pt[i][:, s * 32 : (s + 1) * 32],
                                zloc[s][32 * i : 32 * (i + 1), c * 128 : (c + 1) * 128],
                                id128[32 * i : 32 * (i + 1), 32 * i : 32 * (i + 1)],
                                tile_position=(32 * i, 0),
                            )
                    pst.append(pt)

                if last:
                    for c in range(2):
                        ot = workp.tile([128, 256], f32, name=f"ot{c}", tag="ot")
                        for s in range(2):
                            for i in range(4):
                                k = 4 * s + i
                                nc.scalar.copy(
                                    ot[:, k * 32 : (k + 1) * 32],
                                    pst[c][i][:, s * 32 : (s + 1) * 32],
                                )
                        nc.sync.dma_start(out_dram[c], ot[:])
                    return zloc

                # bf16 casts
                ztl = []
                for s in range(2):
                    t = statep.tile([128, NLOC], bf16, name=f"ztl{rnd}{s}", tag=f"ztl{s}")
                    nc.vector.tensor_copy(t[:], zloc[s][:])
                    ztl.append(t)
                znml = []
                for c in range(2):
                    t = workp.tile([128, 256], bf16, name=f"znml{rnd}{c}", tag=f"znml{c}")
                    for i in range(4):
                        nc.vector.tensor_copy(
                            t[:, i * 64 : (i + 1) * 64], pst[c][i][:, 0:64]
                        )
                    znml.append(t)

                # AllGather both layouts (4 x [128, 256] bf16 = 256KB/rank)
                agin = dramp.tile([4, 128, 256], bf16, name=f"agin{rnd}", tag="agin")
                agout = dramp.tile(
                    [NCORES, 4, 128, 256], bf16,
                    name=f"agout{rnd}", tag="agout", addr_space="Shared",
                )
                for s in range(2):
                    nc.sync.dma_start(agin[s], ztl[s][:])
                for c in range(2):
                    nc.sync.dma_start(agin[2 + c], znml[c][:])
                nc.gpsimd.collective_compute(
                    "AllGather",
                    mybir.AluOpType.bypass,
                    replica_groups=rg,
                    ins=[agin[:].opt()],
                    outs=[agout[:].opt()],
                )
                # reload replicas
                for s in range(2):
                    for r in range(NCORES):
                        nc.sync.dma_start(
                            ZTs[s][:, r * NLOC : (r + 1) * NLOC], agout[r, s]
                        )
                for r in range(NCORES):
                    for c in range(2):
                        blk = r * 2 + c
                        nc.sync.dma_start(
                            Znm[:, blk * 256 : (blk + 1) * 256], agout[r, 2 + c]
                        )
                return zloc, ztl

            # ---- init: Z0 = l2norm(features @ W + b) for local columns ---
            for rep in range(reps):
                _body_once(
                    nc, tc, tile, mybir, rep,
                    featT, wstack, bstack, onesblk, id128, epsb, maskT, ZTs, Znm,
                    statep, workp, psp, psaggp, dramp, out_dram, rg,
                    normalize_and_distribute,
                )

    nc.compile()
    return nc


def _body_once(
    nc, tc, tile, mybir, rep,
    featT, wstack, bstack, onesblk, id128, epsb, maskT, ZTs, Znm,
    statep, workp, psp, psaggp, dramp, out_dram, rg,
    normalize_and_distribute,
):
    f32 = mybir.dt.float32
    bf16 = mybir.dt.bfloat16
    AF = mybir.ActivationFunctionType
    ALU = mybir.AluOpType
    if True:
        if True:
            zsum0 = []
            for s in range(2):
                ip = psp.tile([128, NLOC], f32, name=f"initp{rep}{s}", tag=f"sps{s}")
                for i in range(4):
                    nc.tensor.matmul(
                        ip[32 * i : 32 * (i + 1), :],
                        wstack[:, (4 * s + i) * D : (4 * s + i + 1) * D],
                        featT[:],
                        start=True,
                        stop=True,
                        tile_position=(0, 32 * i),
                    )
                zs = workp.tile([128, NLOC], f32, name=f"zsum0{s}", tag="zsum")
                nc.vector.tensor_scalar(
                    zs[:], ip[:], bstack[:, s : s + 1], None, ALU.add
                )
                zsum0.append(zs)
            zloc, ztl = normalize_and_distribute(zsum0, 0, last=False)

            # ---- iterations ---------------------------------------------
            for it in range(ITERS):
                aggps = [
                    psaggp.tile([128, NLOC], f32, name=f"agg{it}{s}", tag=f"agg{s}")
                    for s in range(2)
                ]
                for blk in range(NBLK):
                    # score tiles: one PSUM bank per row group i; stack s in
                    # column half s. E slot layout: slot(4s+i) = 2i+s.
                    sps = [
                        psp.tile(
                            [128, 2 * NLOC], f32, name=f"sps{it}{blk}{i}", tag=f"sps{i}"
                        )
                        for i in range(4)
                    ]
                    for s in range(2):
                        for i in range(4):
                            nc.tensor.matmul(
                                sps[i][:, s * NLOC : (s + 1) * NLOC],
                                ZTs[s][32 * i : 32 * (i + 1), blk * 128 : (blk + 1) * 128],
                                ztl[s][32 * i : 32 * (i + 1), :],
                                start=True,
                                stop=True,
                                tile_position=(32 * i, 0),
                            )
                    eall = workp.tile([128, K * NLOC], bf16, name=f"eall{it}{blk}", tag="eall")
                    for i in range(4):
                        nc.scalar.activation(
                            eall[:, i * 512 : (i + 1) * 512], sps[i][:], AF.Exp
                        )
                    # denominator tree: 3 ops
                    t1 = workp.tile([128, 1024], bf16, name=f"t1_{it}{blk}", tag="t1")
                    ev = eall[:].rearrange("p (a n) -> p a n", a=4)
                    t1v = t1[:].rearrange("p (a n) -> p a n", a=4)
                    nc.vector.tensor_tensor(
                        t1v, ev[:, :, 0:NLOC], ev[:, :, NLOC : 2 * NLOC], ALU.add
                    )
                    t2 = workp.tile([128, 512], bf16, name=f"t2_{it}{blk}", tag="t2")
                    t1w = t1[:].rearrange("p (a n) -> p a n", a=2)
                    t2v = t2[:].rearrange("p (a n) -> p a n", a=2)
                    nc.vector.tensor_tensor(
                        t2v, t1w[:, :, 0:NLOC], t1w[:, :, NLOC : 2 * NLOC], ALU.add
                    )
                    den = workp.tile([128, NLOC], f32, name=f"den{it}{blk}", tag="den")
                    nc.vector.tensor_tensor(
                        den[:], t2[:, 0:NLOC], t2[:, NLOC : 2 * NLOC], ALU.add
                    )
                    rcp = workp.tile([128, NLOC], f32, name=f"rcp{it}{blk}", tag="rcp")
                    nc.vector.reciprocal_approx_fast(rcp[:], den[:])
                    rmask = workp.tile([128, NLOC], bf16, name=f"rm{it}{blk}", tag="rm")
                    nc.vector.tensor_tensor(
                        rmask[:], rcp[:], maskT[:, blk * NLOC : (blk + 1) * NLOC], ALU.mult
                    )
                    att = workp.tile([128, K * NLOC], bf16, name=f"att{it}{blk}", tag="att")
                    nc.vector.tensor_tensor(
                        att[:].rearrange("p (a n) -> p a n", a=K),
                        eall[:].rearrange("p (a n) -> p a n", a=K),
                        rmask[:, None, :].to_broadcast((128, K, NLOC)),
                        ALU.mult,
                    )
                    for s in range(2):
                        for i in range(4):
                            slot = 2 * i + s  # channel 4s+i in replica layouts
                            nc.tensor.matmul(
                                aggps[s][32 * i : 32 * (i + 1), :],
                                Znm[:, (blk * K + slot) * D : (blk * K + slot + 1) * D],
                                att[:, slot * NLOC : (slot + 1) * NLOC],
                                start=(blk == 0),
                                stop=(blk == NBLK - 1),
                                tile_position=(0, 32 * i),
                                skip_group_check=True,
                            )
                # residual + renorm + redistribute
                zsum = []
                for s in range(2):
                    zs = workp.tile([128, NLOC], f32, name=f"zsum{it}{s}", tag="zsum")
                    nc.vector.tensor_tensor(zs[:], zloc[s][:], aggps[s][:], ALU.add)
                    zsum.append(zs)
                if it == ITERS - 1:
                    normalize_and_distribute(zsum, it + 1, last=True)
                else:
                    zloc, ztl = normalize_and_distribute(zsum, it + 1, last=False)


def _prep_inputs(adj, features, W, b):
    adj = np.asarray(adj)
    features = np.asarray(features, np.float32)
    W = np.asarray(W, np.float32)
    b = np.asarray(b, np.float32)

    wstack = np.ascontiguousarray(W.transpose(1, 0, 2).reshape(IN_DIM, K * D))
    bstack = np.zeros((128, 2), np.float32)
    for s in range(2):
        for i in range(4):
            bstack[32 * i : 32 * (i + 1), s] = b[4 * s + i]
    onesblk = np.zeros((128, 128), np.float32)
    for j in range(4):
        onesblk[32 * j : 32 * (j + 1), 32 * j : 32 * (j + 1)] = 1.0
    id128 = np.eye(128, dtype=np.float32)

    in_maps = []
    for c in range(NCORES):
        rows = slice(c * NLOC, (c + 1) * NLOC)
        featT_loc = np.ascontiguousarray(features[rows].T)
        maskT = (adj[rows].T > 0).astype(np.float32).astype(BF)
        maskT = np.ascontiguousarray(maskT.reshape(NBLK, 128, NLOC))
        in_maps.append(
            {
                "featT_loc": featT_loc,
                "maskT": maskT,
                "wstack": wstack,
                "bstack": bstack,
                "onesblk": onesblk,
                "id128": id128,
            }
        )
    return in_maps


def run(adj, features, W, b, trace=False, **trace_kwargs):
    global _compiled
    if _compiled is None:
        _compiled = _build()
    from concourse import bass_utils

    in_maps = _prep_inputs(adj, features, W, b)
    res = bass_utils.run_bass_kernel_spmd(
        _compiled, in_maps, core_ids=list(range(NCORES)), trace=trace, **trace_kwargs
    )
    outs = [res.results[c]["out"].reshape(NLOC, NLOC) for c in range(NCORES)]
    full = np.concatenate(outs, axis=0)
    return full, res


def kernel(adj, features, W, b):
    full, _ = run(adj, features, W, b, trace=False)
    return full
